# revision 5
# baseline (speedup 1.0000x reference)
"""Baichuan sliding-window GQA attention block on 8 trn2 NeuronCores.

Sharding: data-parallel over batch (2) x tensor-parallel over heads (4).
Core c handles batch b=c//4, head group g=c%4 (q heads 4g..4g+3, kv heads
2g..2g+1). Each core computes qkv projection, RoPE, 2-tap causal conv,
windowed attention and a row-sharded o_proj partial; the host sums the 4
partials per batch.

All on-chip tensors live in a transposed (feature, token) layout so the
tensor engine's contraction (partition) axis lines up without transposes:
  scoresT[k,q] = sum_d kT[d,k] qT[d,q];  outT[d,q] = sum_k v[k,d] probsT[k,q]
V alone is flipped to (token, dim) via PE transposes. Matmuls run as
float32r (full PE rate for moving dim >= 256, fp32 storage).
"""

import numpy as np

B, S, H = 2, 2048, 2048
NH, NKV, HD = 16, 8, 128
WINDOW = 1024
THETA = 100000.0
TP = 4                      # tensor-parallel ways (head groups)
QH = NH // TP               # 4 q heads per core
KVH = NKV // TP             # 2 kv heads per core
NCORES = 8
SCALE = 1.0 / float(np.sqrt(HD))
NEG = -1.0e30

_CACHE = {}


def _build_program():
    import concourse.bacc as bacc
    import concourse.mybir as mybir
    import concourse.tile as tile

    f32 = mybir.dt.float32
    f32r = mybir.dt.float32r
    Exp = mybir.ActivationFunctionType.Exp
    mult = mybir.AluOpType.mult
    add = mybir.AluOpType.add

    nc = bacc.Bacc("TRN2", target_bir_lowering=False, debug=False,
                   enable_asserts=False, num_devices=NCORES)

    hT_d = nc.dram_tensor("hT", [H, S], f32r, kind="ExternalInput")
    wpk_d = nc.dram_tensor("wpk", [H, 1024], f32r, kind="ExternalInput")
    wo_d = nc.dram_tensor("wo", [QH * HD, H], f32r, kind="ExternalInput")
    cs_d = nc.dram_tensor("cs", [128, S], f32, kind="ExternalInput")
    sn_d = nc.dram_tensor("sn", [128, S], f32, kind="ExternalInput")
    cw_d = nc.dram_tensor("cw", [128, 8], f32, kind="ExternalInput")
    msk_d = nc.dram_tensor("msk", [128, 2048], f32, kind="ExternalInput")
    eye_d = nc.dram_tensor("eye", [128, 128], f32, kind="ExternalInput")
    one_d = nc.dram_tensor("one", [128, 128], f32r, kind="ExternalInput")
    yT_d = nc.dram_tensor("yT", [H, S], f32, kind="ExternalOutput")

    NT = S // 256            # 8 token chunks of 256
    NK = H // 128            # 16 contraction tiles

    with tile.TileContext(nc) as tc:
        with (
            tc.tile_pool(name="const", bufs=1) as cp,
            tc.tile_pool(name="persist", bufs=1) as pp,
        ):
            cs_sb = cp.tile([128, S], f32, tag="cs", name="cs")
            sn_sb = cp.tile([128, S], f32, tag="sn", name="sn")
            cw_sb = cp.tile([128, 8], f32, tag="cw", name="cw")
            eye_sb = cp.tile([128, 128], f32, tag="eye", name="eye")
            one_sb = cp.tile([128, 128], f32r, tag="one", name="one")
            nc.sync.dma_start(out=cs_sb[:], in_=cs_d[:, :])
            nc.sync.dma_start(out=sn_sb[:], in_=sn_d[:, :])
            nc.sync.dma_start(out=cw_sb[:], in_=cw_d[:, :])
            nc.sync.dma_start(out=eye_sb[:], in_=eye_d[:, :])
            nc.sync.dma_start(out=one_sb[:], in_=one_d[:, :])

            # persistent across phases
            qpair = [pp.tile([128, 2 * S], f32r, tag=f"qp{i}", name=f"qp{i}") for i in range(KVH)]
            kconv = [pp.tile([128, S], f32r, tag=f"kc{i}", name=f"kc{i}") for i in range(KVH)]
            vt = [[pp.tile([128, 128], f32r, tag=f"vt{i}_{j}", name=f"vt{i}_{j}") for j in range(NK)]
                  for i in range(KVH)]

            # ---- phase B1: q/k projection + rope, then k conv ----
            with (
                tc.tile_pool(name="b1", bufs=2) as bp,
                tc.tile_pool(name="b1w", bufs=1) as bw,
                tc.tile_pool(name="b1ps", bufs=4, space="PSUM") as psb,
            ):
                wqk = [bw.tile([128, 768], f32r, tag=f"wqk{k}", name=f"wqk{k}") for k in range(NK)]
                for k in range(NK):
                    nc.sync.dma_start(out=wqk[k][:],
                                      in_=wpk_d[k * 128:(k + 1) * 128, 0:768])
                kT = [bw.tile([128, S], f32, tag=f"kT{i}", name=f"kT{i}") for i in range(KVH)]
                for t in range(NT):
                    hts = []
                    for k in range(NK):
                        ht = bp.tile([128, 256], f32r, tag=f"ht{k}", name=f"ht{k}")
                        nc.sync.dma_start(
                            out=ht[:],
                            in_=hT_d[k * 128:(k + 1) * 128, t * 256:(t + 1) * 256])
                        hts.append(ht)
                    csl = cs_sb[:, t * 256:(t + 1) * 256]
                    snl = sn_sb[:, t * 256:(t + 1) * 256]
                    for col in range(6):
                        ps = psb.tile([128, 256], f32, tag="qkps", name="qkps")
                        for k in range(NK):
                            nc.tensor.matmul(
                                ps[:],
                                wqk[k][:, col * 128:(col + 1) * 128],
                                hts[k][:],
                                start=(k == 0), stop=(k == NK - 1))
                        e1 = bp.tile([128, 256], f32, tag="e1", name="e1")
                        e2 = psb.tile([128, 256], f32, tag="e2", name="e2", bufs=2)
                        nc.vector.tensor_mul(e1[:], ps[:], csl)
                        nc.vector.tensor_mul(e2[:], ps[:], snl)
                        if col < 4:
                            dest = qpair[col // 2]
                            off = (col % 2) * S + t * 256
                        else:
                            dest = kT[col - 4]
                            off = t * 256
                        nc.vector.tensor_sub(dest[0:64, off:off + 256],
                                             e1[0:64, :], e2[64:128, :])
                        nc.vector.tensor_add(dest[64:128, off:off + 256],
                                             e2[0:64, :], e1[64:128, :])
                        del e2
                # k conv: kc[t] = w0*k[t-1] + w1*k[t]
                for i in range(KVH):
                    tmp = bw.tile([128, S], f32, tag="ctmp", name="ctmp")
                    nc.vector.tensor_scalar_mul(tmp[:], kT[i][:],
                                                cw_sb[:, 2 * i + 1:2 * i + 2])
                    nc.vector.scalar_tensor_tensor(
                        kconv[i][:, 1:S], kT[i][:, 0:S - 1],
                        cw_sb[:, 2 * i:2 * i + 1], tmp[:, 1:S], mult, add)
                    nc.vector.tensor_copy(kconv[i][:, 0:1], tmp[:, 0:1])

            # ---- phase B2: v projection, conv, transpose to (token, dim) ----
            with (
                tc.tile_pool(name="b2", bufs=2) as bp2,
                tc.tile_pool(name="b2w", bufs=1) as bw2,
                tc.tile_pool(name="b2ps", bufs=4, space="PSUM") as psb2,
                tc.tile_pool(name="b2tp", bufs=2, space="PSUM") as pst,
            ):
                wv = [bw2.tile([128, 256], f32r, tag=f"wv{k}", name=f"wv{k}") for k in range(NK)]
                for k in range(NK):
                    nc.sync.dma_start(out=wv[k][:],
                                      in_=wpk_d[k * 128:(k + 1) * 128, 768:1024])
                vT = [bw2.tile([128, S], f32, tag=f"vT{i}", name=f"vT{i}") for i in range(KVH)]
                for t in range(NT):
                    hts = []
                    for k in range(NK):
                        ht = bp2.tile([128, 256], f32r, tag=f"hu{k}", name=f"hu{k}")
                        nc.sync.dma_start(
                            out=ht[:],
                            in_=hT_d[k * 128:(k + 1) * 128, t * 256:(t + 1) * 256])
                        hts.append(ht)
                    for i in range(KVH):
                        ps = psb2.tile([128, 256], f32, tag="vps", name="vps")
                        for k in range(NK):
                            nc.tensor.matmul(
                                ps[:],
                                wv[k][:, i * 128:(i + 1) * 128],
                                hts[k][:],
                                start=(k == 0), stop=(k == NK - 1))
                        nc.scalar.copy(vT[i][:, t * 256:(t + 1) * 256], ps[:])
                for i in range(KVH):
                    vc = bw2.tile([128, S], f32, tag="vc", name="vc")
                    tmp = bw2.tile([128, S], f32, tag="ctmp2", name="ctmp2")
                    nc.vector.tensor_scalar_mul(tmp[:], vT[i][:],
                                                cw_sb[:, 4 + 2 * i + 1:4 + 2 * i + 2])
                    nc.vector.scalar_tensor_tensor(
                        vc[:, 1:S], vT[i][:, 0:S - 1],
                        cw_sb[:, 4 + 2 * i:4 + 2 * i + 1], tmp[:, 1:S], mult, add)
                    nc.vector.tensor_copy(vc[:, 0:1], tmp[:, 0:1])
                    for j in range(NK):
                        tp = pst.tile([128, 128], f32, tag="vtp", name="vtp")
                        nc.tensor.transpose(tp[:], vc[:, j * 128:(j + 1) * 128],
                                            eye_sb[:])
                        nc.vector.tensor_copy(vt[i][j][:], tp[:])

            # ---- phase E: banded attention;  phase F: o_proj partial ----
            with (
                tc.tile_pool(name="att", bufs=1) as ap,
                tc.tile_pool(name="atw", bufs=3) as aw,
            ):
                msk_sb = ap.tile([128, 2048], f32, tag="msk", name="msk")
                nc.sync.dma_start(out=msk_sb[:], in_=msk_d[:, :])
                wo_sb = [ap.tile([128, H], f32r, tag=f"wo{d}", name=f"wo{d}") for d in range(QH)]
                for d in range(QH):
                    nc.sync.dma_start(out=wo_sb[d][:],
                                      in_=wo_d[d * 128:(d + 1) * 128, :])
                attn = [ap.tile([128, S], f32r, tag=f"at{h}", name=f"at{h}") for h in range(QH)]

                with (
                    tc.tile_pool(name="eps_sc", bufs=2, space="PSUM") as pss,
                    tc.tile_pool(name="eps_pv", bufs=2, space="PSUM") as psv,
                    tc.tile_pool(name="eps_sm", bufs=2, space="PSUM") as psm,
                ):
                  for i in range(KVH):
                    for qi in range(NT):
                        qc = qi * 256
                        jstart = max(0, qc // 128 - 8)
                        jend = qc // 128 + 1
                        ps_o = psv.tile([128, 512], f32, tag="pv", name="pv")
                        ps_s = psm.tile([1, 512], f32, tag="sm", name="sm")
                        for j in range(jstart, jend + 1):
                            ps_sc = pss.tile([128, 512], f32, tag="sc", name="sc")
                            lhs = kconv[i][:, j * 128:(j + 1) * 128]
                            nc.tensor.matmul(
                                ps_sc[:, 0:256], lhs,
                                qpair[i][:, qc:qc + 256],
                                start=True, stop=True)
                            nc.tensor.matmul(
                                ps_sc[:, 256:512], lhs,
                                qpair[i][:, S + qc:S + qc + 256],
                                start=True, stop=True)
                            mt = {0: 0, -128: 1, 896: 2, 1024: 3}.get(qc - j * 128)
                            pb = aw.tile([128, 512], f32r, tag="pb", name="pb")
                            if mt is None:
                                nc.scalar.activation(pb[:], ps_sc[:], Exp,
                                                     bias=0.0, scale=SCALE)
                            else:
                                tm = aw.tile([128, 512], f32, tag="tm", name="tm")
                                nc.vector.tensor_add(
                                    tm[:], ps_sc[:],
                                    msk_sb[:, mt * 512:(mt + 1) * 512])
                                nc.scalar.activation(pb[:], tm[:], Exp,
                                                     bias=0.0, scale=SCALE)
                            nc.tensor.matmul(ps_o[:], vt[i][j][:],
                                             pb[:],
                                             start=(j == jstart), stop=(j == jend))
                            nc.tensor.matmul(ps_s[:], one_sb[:, 0:1],
                                             pb[:],
                                             start=(j == jstart), stop=(j == jend))
                        rsum = aw.tile([1, 512], f32, tag="rs", name="rs")
                        nc.vector.reciprocal(rsum[:], ps_s[:])
                        rb = aw.tile([128, 512], f32, tag="rb", name="rb")
                        nc.gpsimd.partition_broadcast(rb[:], rsum[:])
                        nc.vector.tensor_mul(attn[2 * i][:, qc:qc + 256],
                                             ps_o[:, 0:256], rb[:, 0:256])
                        nc.vector.tensor_mul(attn[2 * i + 1][:, qc:qc + 256],
                                             ps_o[:, 256:512], rb[:, 256:512])

                with tc.tile_pool(name="fps", bufs=4, space="PSUM") as psf:
                    for oc in range(NK):
                        for t4 in range(4):
                            ps_y = psf.tile([128, 512], f32, tag="y", name="y")
                            for d in range(QH):
                                nc.tensor.matmul(
                                    ps_y[:],
                                    wo_sb[d][:, oc * 128:(oc + 1) * 128],
                                    attn[d][:, t4 * 512:(t4 + 1) * 512],
                                    start=(d == 0), stop=(d == QH - 1))
                            yb = aw.tile([128, 512], f32, tag="yb", name="yb")
                            if (oc + t4) % 2 == 0:
                                nc.vector.tensor_copy(yb[:], ps_y[:])
                            else:
                                nc.scalar.copy(yb[:], ps_y[:])
                            nc.sync.dma_start(
                                out=yT_d[oc * 128:(oc + 1) * 128,
                                         t4 * 512:(t4 + 1) * 512],
                                in_=yb[:])

    nc.finalize()
    return nc


def _host_inputs(hidden, W_pack, W_o, conv_k, conv_v):
    """Per-core input maps."""
    pos = np.arange(S, dtype=np.float64)
    inv_freq = 1.0 / (THETA ** (np.arange(0, HD, 2, dtype=np.float64) / HD))
    freqs = np.outer(pos, inv_freq)                       # (S, 64)
    cos = np.cos(freqs).T.astype(np.float32)              # (64, S)
    sin = np.sin(freqs).T.astype(np.float32)
    cs = np.concatenate([cos, cos], axis=0)               # (128, S)
    sn = np.concatenate([sin, sin], axis=0)

    kk = np.arange(128)[:, None]
    qq = np.arange(256)[None, :]
    def double(m):
        return np.concatenate([m, m], axis=1).astype(np.float32)
    t0 = double(np.where(kk <= qq, 0.0, NEG))             # delta = 0
    tm128 = double(np.where(kk <= qq - 128, 0.0, NEG))    # delta = -128
    w896 = double(np.where(qq - kk < 128, 0.0, NEG))      # delta = 896
    w1024 = double(np.where(qq < kk, 0.0, NEG))           # delta = 1024
    msk = np.concatenate([t0, tm128, w896, w1024], axis=1)  # (128, 2048)

    eye = np.eye(128, dtype=np.float32)
    one = np.ones((128, 128), dtype=np.float32)

    in_maps = []
    for c in range(NCORES):
        b, g = c // TP, c % TP
        hT = np.ascontiguousarray(hidden[b].T).astype(np.float32)   # (H, S)
        wq = W_pack[:, g * 512:(g + 1) * 512]
        wk = W_pack[:, NH * HD + 2 * g * 128: NH * HD + (2 * g + 2) * 128]
        wv = W_pack[:, NH * HD + NKV * HD + 2 * g * 128:
                    NH * HD + NKV * HD + (2 * g + 2) * 128]
        wpk = np.ascontiguousarray(
            np.concatenate([wq, wk, wv], axis=1)).astype(np.float32)
        wo = np.ascontiguousarray(
            W_o[g * 512:(g + 1) * 512, :]).astype(np.float32)
        cwv = np.empty(8, np.float32)
        for i in range(KVH):
            cwv[2 * i] = conv_k[2 * g + i, 0]
            cwv[2 * i + 1] = conv_k[2 * g + i, 1]
            cwv[4 + 2 * i] = conv_v[2 * g + i, 0]
            cwv[4 + 2 * i + 1] = conv_v[2 * g + i, 1]
        cw = np.broadcast_to(cwv, (128, 8)).copy()
        in_maps.append({
            "hT": hT, "wpk": wpk, "wo": wo, "cs": cs, "sn": sn,
            "cw": cw, "msk": msk, "eye": eye, "one": one,
        })
    return in_maps


def run_cores(in_maps, trace=False, **kw):
    from concourse.bass_utils import run_bass_kernel_spmd
    if "nc" not in _CACHE:
        _CACHE["nc"] = _build_program()
    return run_bass_kernel_spmd(_CACHE["nc"], in_maps, list(range(NCORES)),
                                trace=trace, **kw)


def kernel(hidden, W_pack, W_o, conv_k, conv_v):
    hidden = np.asarray(hidden, np.float32)
    W_pack = np.asarray(W_pack, np.float32)
    W_o = np.asarray(W_o, np.float32)
    conv_k = np.asarray(conv_k, np.float32)
    conv_v = np.asarray(conv_v, np.float32)
    in_maps = _host_inputs(hidden, W_pack, W_o, conv_k, conv_v)
    res = run_cores(in_maps)
    out = np.zeros((B, S, H), np.float32)
    for c in range(NCORES):
        b = c // TP
        out[b] += res.results[c]["yT"].T
    return out
